# revision 37
# baseline (speedup 1.0000x reference)
"""Multi-head causal self-attention (B=2, T=4096, C=768, H=12, D=64) on 8 NeuronCores.

Sharding: core c handles batch b = c // 4 and a group of 3 heads (c % 4).
Each core runs a fused flash-attention pipeline per 512-column tq chunk:
Q/K projection (3 weight chunks) -> V projection in natural [key, d] layout
via swapped matmuls (lhsT = x.T key-block) -> streaming softmax(QK^T)V ->
output projection, producing a partial (pre-bias) out.T [768, 4096] in bf16.
The host sums the 4 partials per batch and adds b_proj + b_v @ W_proj (the
V bias commutes through softmax normalization, so V is projected bias-free).

All score/attnV matmuls run in bf16 (scores are O(+-6); bf16 q/k adds ~0.3%
noise). Softmax skips max-subtraction; the denominator comes from an
appended ones-column in V. exp() is split across engines: diagonal (masked)
pairs use exact exp on the Activation engine with gpsimd mask-multiplies;
off-diagonal pairs alternate between Activation-exp and a Schraudolph
approximation on the Vector engine computed directly in int16
(int16(x*A/2^16 + B/2^16) is the top half of the fp32 exp bit pattern, i.e.
bf16 -- read back via a contiguous bitcast, full-rate for the PE).

The per-core QKV weight columns live in 3 chunks of 128: [Q0|Q1], [K0|K1],
[Q2|K2]. Q0/Q1 share qt slot 0 (zero-padded KT halves kill cross terms);
Q2 uses slot 1 (upper half zero). K2's projection lands on partitions
64:128 but must contract against Q2 on 0:64, so it is staged into KT[2]'s
(unused) upper rows and partition-shifted down by an SBUF->SBUF DMA --
engines cannot cross partition lanes, DMA can.

Normalization: -1/den per head via a bit-affine reciprocal seed plus one
Newton step on the Vector engine (sign absorbed by negating W_proj
host-side), written as fp16 rows of a shared tile; one ones-weighted fp16
matmul per ot slot broadcasts them across partitions.
"""

from contextlib import ExitStack

import numpy as np

import concourse.bass as bass
import concourse.tile as tile
from concourse import bacc
from concourse import mybir
from concourse._compat import with_exitstack
from concourse.bass_utils import run_bass_kernel_spmd

F32 = mybir.dt.float32
F16 = mybir.dt.float16
BF16 = mybir.dt.bfloat16
I16 = mybir.dt.int16
I32 = mybir.dt.int32
EXP = mybir.ActivationFunctionType.Exp
IDENT = mybir.ActivationFunctionType.Identity
MULT = mybir.AluOpType.mult
ADD = mybir.AluOpType.add

B, T, C = 2, 4096, 768
H, D = 12, 64
NCORES = 8
HPC = 3           # heads per core
GPB = NCORES // B  # head-group cores per batch (4)
TQ = 512          # tq chunk width
NJ = T // TQ      # 8
TKB = 128         # tk block
NB = T // TKB     # 32
KC = C // 128     # 6 contraction chunks for the projections
NQK = 3 * 128     # per-core Q/K weight columns
NV = HPC * D      # 192 per-core V weight columns
SCALE = 1.0 / np.sqrt(D)

# Schraudolph exp: exp(x) ~= bitcast_f32(int32(x * 2^23/ln2 + (127<<23) - CADJ))
# computed directly in the top 16 bits (== bf16) via an int16 tensor_scalar.
SCH_A = 12102203.0
SCH_CADJ = 366393.0
SCH_B = float((127 << 23)) - SCH_CADJ
RECIP_MAGIC = 0x7EF311C3  # bit-affine 1/x seed; one Newton step -> +-0.26%

import os
SCH_8THS = int(os.environ.get("SCH_8THS", "4"))  # ip%8 < this -> DVE exp
PIPE_ENABLE = os.environ.get("PIPE_ENABLE", "1") == "1"
# priority boosts (0=off). CAUTION: boosting an op whose dependency is
# produced late on another engine reorders it ahead of that engine's
# consumers and deadlocks the in-order streams.
HP_QKV = int(os.environ.get("HP_QKV", "0"))
HP_NORM = int(os.environ.get("HP_NORM", "0"))
WARMUP_MMS = int(os.environ.get("WARMUP_MMS", "80"))

# Vp block layout [128 keys, 128 cols]: per head, V data occupies VCOL..VCOL+64
# and a ones-column at ONES_COL supplies the softmax denominator. attnV output
# partitions = lhsT free index, so heads 0/1 land their O at rows 64:128 and
# head 2 at rows 0:64 -> heads pack into two 128-row ot slots for the output
# projection (slot0 = h2 rows 0:64 + h0 rows 64:128; slot1 = h1 rows 64:128).
VCOL = {0: 64, 1: 64, 2: 0}
ONES_COL = {0: 32, 1: 0, 2: 64}  # distinct + 32-aligned: den rows 32/0/64
LHS_W = {0: 128, 1: 128, 2: 65}  # attnV lhsT free width
DEN_ROW = {0: 32, 1: 0, 2: 64}  # pso row holding the denominator
O_ROW = {0: 64, 1: 64, 2: 0}    # pso/ot row base of the 64 output dims
OT_SLOT = {0: 0, 1: 1, 2: 0}    # ot slot per head
QT_SLOT = {0: 0, 1: 0, 2: 1}    # qt slot per head
NSLOT = 2


def _proj(nc, ps_misc, stp, wp_sb, outT_r, ot, j, qeng):
    jsl = slice(j * TQ, (j + 1) * TQ)
    for m in range(KC):
        ps3 = ps_misc.tile([128, TQ], F32, tag="misc", name="ps3")
        for sl in range(NSLOT):
            nc.tensor.matmul(
                ps3[:],
                lhsT=wp_sb[:, sl, m * 128:(m + 1) * 128],
                rhs=ot[:, sl, :],
                start=(sl == 0),
                stop=(sl == NSLOT - 1),
            )
        st = stp.tile([128, TQ], BF16, tag="st", name="st")
        if m % 2 == 0:
            nc.vector.tensor_copy(st[:], ps3[:])
        else:
            nc.scalar.copy(st[:], ps3[:])
        # mid-kernel stores ride the gpsimd DGE queue so their triggers
        # (waiting on st copies) never head-of-line block the sync queue's
        # xt prefetches; the final chunk uses sync (prompt completion).
        qeng.dma_start(outT_r[:, m, jsl], st[:])


@with_exitstack
def _mhsa_body(ctx: ExitStack, tc: tile.TileContext, t):
    from contextlib import nullcontext

    nc = tc.nc

    def hp(offset):
        return tc.high_priority(offset) if offset else nullcontext()

    norm_cb = None  # deferred per-chunk normalize (flushed next chunk)
    xT_r = t["xT"].rearrange("(kc p) t -> p kc t", p=128)
    outT_r = t["outT"].rearrange("(mo p) t -> p mo t", p=128)
    wqk_r = t["wqk"].rearrange("(kc p) m -> p kc m", p=128)
    wv_r = t["wv"].rearrange("(kc p) m -> p kc m", p=128)

    const = ctx.enter_context(tc.tile_pool(name="const", bufs=1))
    persist = ctx.enter_context(tc.tile_pool(name="persist", bufs=1))
    xpool = ctx.enter_context(tc.tile_pool(name="xpool", bufs=2))
    ptp = ctx.enter_context(tc.tile_pool(name="ptp", bufs=4))
    ptip = ctx.enter_context(tc.tile_pool(name="ptip", bufs=4))
    stp = ctx.enter_context(tc.tile_pool(name="stp", bufs=3))
    lrp = ctx.enter_context(tc.tile_pool(name="lrp", bufs=4))
    otmpp = ctx.enter_context(tc.tile_pool(name="otmpp", bufs=4))

    ps_pair = ctx.enter_context(tc.tile_pool(name="ps_pair", bufs=2, space="PSUM"))
    ps_o = ctx.enter_context(tc.tile_pool(name="ps_o", bufs=2, space="PSUM"))
    ps_misc = ctx.enter_context(tc.tile_pool(name="ps_misc", bufs=2, space="PSUM"))

    # chunk 0's x slices go first: the first Q/K matmul needs xt(kc0)+wqk(kc0)
    xt0 = xpool.tile([128, KC, TQ], BF16, tag="xt")
    wqk_sb = const.tile([128, KC, NQK], BF16)
    wv_sb = const.tile([128, KC, NV], BF16)
    for kc in range(KC):
        nc.sync.dma_start(xt0[:, kc, :], xT_r[:, kc, 0:TQ])
        nc.sync.dma_start(wqk_sb[:, kc, :], wqk_r[:, kc, :])
    bias_sb = const.tile([128, 3], F32)
    nc.sync.dma_start(bias_sb[:], t["bqk"].rearrange("m p -> p m"))
    for kc in range(KC):  # V weights: needed only mid-chunk, loaded after
        nc.sync.dma_start(wv_sb[:, kc, :], wv_r[:, kc, :])
    mask_sb = const.tile([128, 1280], BF16)
    nc.scalar.dma_start(mask_sb[:], t["masks"])
    # ones_sb: fp16 lhsT patterns for the -1/den broadcast matmuls.
    # cols 0:128 = slot0 (row 32 -> out 64:128 (h0), row 64 -> out 0:64 (h2)),
    # cols 128:256 = slot1 (row 0 -> out 64:128 (h1)).
    ones_sb = const.tile([128, 256], F16)
    nc.gpsimd.memset(ones_sb[:], 0.0)
    nc.gpsimd.memset(ones_sb[32:33, 64:128], 1.0)
    nc.gpsimd.memset(ones_sb[64:65, 0:64], 1.0)
    nc.gpsimd.memset(ones_sb[0:1, 192:256], 1.0)
    wp_sb = const.tile([128, NSLOT, C], BF16)
    nc.scalar.dma_start(wp_sb[:], t["wproj"].rearrange("h p m -> p h m"))

    KT = [persist.tile([128, T], BF16, tag=f"KT{h}", name=f"KT{h}") for h in range(HPC)]
    # V in natural [key, d] layout: h0/h1 share a merged tile (same VCOL) so
    # one DVE copy per key-block covers both; h2 has its own (VCOL 0).
    Vp01 = persist.tile([128, NB, 2, 128], BF16, tag="Vp01", name="Vp01")
    Vp2 = persist.tile([128, NB, 128], BF16, tag="Vp2", name="Vp2")
    qts = [
        persist.tile([128, NSLOT, TQ], BF16, tag=f"qt{i}", name=f"qt{i}")
        for i in range(2)
    ]
    ots = [
        persist.tile([128, NSLOT, TQ], BF16, tag=f"ot{i}", name=f"ot{i}")
        for i in range(2)
    ]
    # den staging rows (h1=0, h0=32, h2=64) for one batched reciprocal chain
    # per chunk; filler rows hold 1.0 so the chain stays finite on them.
    dent = persist.tile([65, TQ], F32, tag="dent", name="dent")
    lrall = persist.tile([65, TQ], F16, tag="lrall", name="lrall")

    # (KT[0]/KT[1] pad halves need no zeroing: the row-tiled score matmuls
    # contract only each head's 64 data rows)
    nc.gpsimd.memset(Vp01[:, :, :, 0:64], 0.0)
    nc.gpsimd.memset(Vp01[:, :, 0, 32:33], 1.0)
    nc.gpsimd.memset(Vp01[:, :, 1, 0:1], 1.0)
    nc.gpsimd.memset(Vp2[:, :, 64:65], 1.0)
    nc.gpsimd.memset(dent[:], 1.0)
    for i in range(2):
        nc.gpsimd.memset(qts[i][64:128, 1, :], 0.0)
        nc.gpsimd.memset(ots[i][0:64, 1, :], 0.0)

    # PE warmup: dummy matmuls during the initial input-DMA wait keep the
    # HAM activity monitor busy so the clock gate opens (1.2 -> 2.4 GHz)
    # before the first real matmul lands (~3.4us of sustained activity).
    wu = const.tile([128, 256], BF16)
    nc.gpsimd.memset(wu[:], 0.0)
    wups = ps_misc.tile([128, TQ], F32, tag="misc", name="wups")
    for _ in range(WARMUP_MMS):
        nc.tensor.matmul(
            wups[:, 0:256], lhsT=wu[:, 0:128], rhs=wu[:, 0:256],
            start=True, stop=True,
        )

    for j in range(NJ):
        jsl = slice(j * TQ, (j + 1) * TQ)

        # ---- Q/K/V projection for this tq chunk ----
        if j == 0:
            xt = xt0
        else:
            xt = xpool.tile([128, KC, TQ], BF16, tag="xt")
            for kc in range(KC):
                nc.sync.dma_start(xt[:, kc, :], xT_r[:, kc, jsl])
        qt = qts[j % 2]

        # W-chunk order: [Q2|K2] first (K2's shift DMA latency), then
        # [Q0|Q1] (qt slot0 gates h0/h1 score starts), V key-blocks, then
        # [K0|K1] last (K blocks gate only each head's diagonal pairs).
        for m in (2, 0):
            ps = ps_misc.tile([128, TQ], F32, tag="misc", name="psqk")
            for kc in range(KC):
                nc.tensor.matmul(
                    ps[:],
                    lhsT=wqk_sb[:, kc, m * 128:(m + 1) * 128],
                    rhs=xt[:, kc, :],
                    start=(kc == 0),
                    stop=(kc == KC - 1),
                )
            with hp(HP_QKV):
                if m == 0:  # [Q0|Q1] -> qt slot0, one full-height biased copy
                    nc.scalar.activation(
                        out=qt[:, 0, :], in_=ps[:], func=IDENT,
                        bias=bias_sb[:, 0:1],
                    )
                else:  # [Q2|K2]
                    nc.vector.tensor_scalar(
                        out=qt[0:64, 1, :], in0=ps[0:64, :],
                        scalar1=bias_sb[0:64, 2:3], scalar2=None, op0=ADD,
                    )
                    # K2: stage into KT[2] rows 64:128 (qt slot1's zero upper
                    # half makes them don't-care for scores), then partition-
                    # shift down with an SBUF->SBUF DMA.
                    nc.scalar.activation(
                        out=KT[2][64:128, jsl], in_=ps[64:128, :], func=IDENT,
                        bias=bias_sb[64:128, 2:3],
                    )
                    nc.gpsimd.dma_start(KT[2][0:64, jsl], KT[2][64:128, jsl])

        for s in range(4):  # V projection, natural [key, d] layout
            psv = ps_misc.tile([128, TQ], F32, tag="misc", name="psv")
            for kc in range(KC):
                nc.tensor.matmul(
                    psv[:, 0:NV],
                    lhsT=xt[:, kc, s * 128:(s + 1) * 128],
                    rhs=wv_sb[:, kc, :],
                    start=(kc == 0),
                    stop=(kc == KC - 1),
                )
            with hp(HP_QKV):
                nc.vector.tensor_copy(
                    out=Vp01[:, 4 * j + s, :, 64:128], in_=psv[:, 0:128]
                )
                nc.vector.tensor_copy(
                    out=Vp2[:, 4 * j + s, 0:64], in_=psv[:, 128:192]
                )

        ps = ps_misc.tile([128, TQ], F32, tag="misc", name="psk")
        for kc in range(KC):
            nc.tensor.matmul(
                ps[:],
                lhsT=wqk_sb[:, kc, 128:256],
                rhs=xt[:, kc, :],
                start=(kc == 0),
                stop=(kc == KC - 1),
            )
        with hp(HP_QKV):
            nc.vector.tensor_scalar(
                out=KT[0][0:64, jsl], in0=ps[0:64, :],
                scalar1=bias_sb[0:64, 1:2], scalar2=None, op0=ADD,
            )
            nc.scalar.activation(
                out=KT[1][64:128, jsl], in_=ps[64:128, :], func=IDENT,
                bias=bias_sb[64:128, 1:2],
            )

        # ---- streaming attention for this tq chunk ----
        ot = ots[j % 2]
        otmp0 = otmpp.tile([128, TQ], F32, tag="otmp", name="otmp0")
        otmp1 = otmpp.tile([128, TQ], F32, tag="otmp", name="otmp1")
        nblk = 4 * j + 4
        npair = nblk // 2
        def pair_geom(ip):
            i0, i1 = 2 * ip, 2 * ip + 1
            # column trim offsets: block i only contributes to tq columns
            # >= 128*(i-4j) within this chunk
            offs = [max(0, 128 * (i - 4 * j)) for i in (i0, i1)]
            ns = [TQ - o for o in offs]
            return (i0, i1), offs, ns, [0, ns[0]], ns[0] + ns[1]

        def exp_act(pss, w):
            pt = ptp.tile([128, 2 * TQ], BF16, tag="pt")
            nc.scalar.activation(
                out=pt[:, :w], in_=pss[:, :w], func=EXP, scale=SCALE
            )
            return pt

        def exp_split(pss):
            # off-diag pair (two full 512 blocks): block0 exact exp on ACT
            # concurrently with block1 Schraudolph on DVE -- halves the exp
            # latency per pair so scores(ip+1) never starve the PE.
            pt = ptp.tile([128, 2 * TQ], BF16, tag="pt")
            nc.scalar.activation(
                out=pt[:, 0:TQ], in_=pss[:, 0:TQ], func=EXP, scale=SCALE
            )
            pt16 = ptip.tile([128, 2 * TQ], I16, tag="pt16")
            nc.vector.tensor_scalar(
                out=pt16[:, TQ:2 * TQ], in0=pss[:, TQ:2 * TQ],
                scalar1=SCH_A * SCALE / 65536.0, scalar2=SCH_B / 65536.0,
                op0=MULT, op1=ADD,
            )
            ptb = pt16.bitcast(BF16)
            return lambda s, n: (pt[:, s:s + n] if s < TQ
                                 else ptb[:, s:s + n])

        def exp_diag(pss, w):
            pt = exp_act(pss, w)
            moff = 0 if w == 896 else 896
            nc.vector.tensor_mul(
                pt[:, :w], pt[:, :w], mask_sb[:, moff:moff + w]
            )
            return lambda s, n: pt[:, s:s + n]

        # ---- head 2 alone (qt slot1; upper contraction half is zero) ----
        # software pipeline by one pair: emit scores(ip) one pair ahead of
        # attnV(ip) so the PE always has exp-independent work queued.
        pso2 = ps_o.tile([128, TQ], F32, tag="pso", name="pso2")
        pending = None  # (rhs, offs, ns, starts, (i0, i1))
        for ip in range(npair + 1):
            if ip < npair:
                blks, offs, ns, starts, w = pair_geom(ip)
                pss = ps_pair.tile([128, 2 * TQ], F32, tag="pss")
                for n, i in enumerate(blks):
                    nc.tensor.matmul(
                        pss[:, starts[n]:starts[n] + ns[n]],
                        lhsT=KT[2][:, i * 128:(i + 1) * 128],
                        rhs=qt[:, 1, offs[n]:TQ],
                        start=True,
                        stop=True,
                    )
                if ip - 2 * j >= 0:
                    rhs_of = exp_diag(pss, w)
                else:
                    rhs_of = exp_split(pss)
                cur = (rhs_of, offs, ns, starts, blks)
            else:
                cur = None
            if not PIPE_ENABLE:
                pending, cur = cur, None
            if pending is not None:
                (p_rhs, p_offs, p_ns, p_starts, p_blks) = pending
                for n, i in enumerate(p_blks):
                    nc.tensor.matmul(
                        pso2[0:65, p_offs[n]:TQ],
                        lhsT=Vp2[:, i, 0:65],
                        rhs=p_rhs(p_starts[n], p_ns[n]),
                        start=(i == 0),
                        stop=(i == nblk - 1),
                    )
            pending = cur

        if norm_cb is not None:
            # previous chunk's deferred normalize: flushed after h2's pair
            # emission (which covers the chain latency) and before h2's
            # dent write (the chain reads all dent rows).
            norm_cb()
            norm_cb = None
        nc.vector.tensor_copy(otmp0[0:64, :], pso2[0:64, :])
        nc.scalar.copy(dent[64:65, :], pso2[64:65, :])

        # ---- heads 0 and 1 jointly: their K/Q data live in complementary
        # 64-partition halves, so each score pair runs as two concurrent
        # 64-deep row-tiled matmuls (tile rows 0:64 for h0, 64:128 for h1).
        pso0 = ps_o.tile([128, TQ], F32, tag="pso", name="pso0")
        pso1 = ps_o.tile([128, TQ], F32, tag="pso", name="pso1")
        pending = None
        for ip in range(npair + 1):
            if ip < npair:
                blks, offs, ns, starts, w = pair_geom(ip)
                pss0 = ps_pair.tile([128, 2 * TQ], F32, tag="pss", name="pss0")
                pss1 = ps_pair.tile([128, 2 * TQ], F32, tag="pss", name="pss1")
                for n, i in enumerate(blks):
                    nc.tensor.matmul(
                        pss0[:, starts[n]:starts[n] + ns[n]],
                        lhsT=KT[0][0:64, i * 128:(i + 1) * 128],
                        rhs=qt[0:64, 0, offs[n]:TQ],
                        start=True,
                        stop=True,
                    )
                    nc.tensor.matmul(
                        pss1[:, starts[n]:starts[n] + ns[n]],
                        lhsT=KT[1][64:128, i * 128:(i + 1) * 128],
                        rhs=qt[64:128, 0, offs[n]:TQ],
                        start=True,
                        stop=True,
                    )
                if ip - 2 * j >= 0:  # diagonal: exact exp + masks, both heads
                    r0f = exp_diag(pss0, w)
                    r1f = exp_diag(pss1, w)
                else:
                    r0f = exp_split(pss0)
                    r1f = exp_split(pss1)
                cur = ((r0f, r1f), offs, ns, starts, blks)
            else:
                cur = None
            if not PIPE_ENABLE:
                pending, cur = cur, None
            if pending is not None:
                ((p0, p1), p_offs, p_ns, p_starts, p_blks) = pending
                for n, i in enumerate(p_blks):
                    nc.tensor.matmul(
                        pso0[:, p_offs[n]:TQ],
                        lhsT=Vp01[:, i, 0, :],
                        rhs=p0(p_starts[n], p_ns[n]),
                        start=(i == 0),
                        stop=(i == nblk - 1),
                    )
                    nc.tensor.matmul(
                        pso1[:, p_offs[n]:TQ],
                        lhsT=Vp01[:, i, 1, :],
                        rhs=p1(p_starts[n], p_ns[n]),
                        start=(i == 0),
                        stop=(i == nblk - 1),
                    )
            pending = cur

        # copy O rows to SBUF (releases the PSUM banks) and stage den rows
        # into the shared dent tile (h1->0, h0->32, h2->64); the reciprocal
        # chain runs once per chunk, batched.
        nc.vector.tensor_copy(otmp0[64:128, :], pso0[64:128, :])
        nc.scalar.copy(dent[32:33, :], pso0[32:33, :])
        nc.vector.tensor_copy(otmp1[64:128, :], pso1[64:128, :])
        nc.vector.tensor_copy(dent[0:1, :], pso1[0:1, :])

        # deferred normalize for this chunk: one batched reciprocal chain
        # -1/den over dent rows {0,32,64} (filler rows hold 1.0), fp16
        # ones-weighted broadcast matmuls per ot slot, then the final muls.
        def norm_chunk(ot=ot, otmp0=otmp0, otmp1=otmp1):
            with hp(HP_NORM):
                r0i = lrp.tile([65, TQ], I32, tag="r0i")
                nc.vector.tensor_scalar(
                    out=r0i[:], in0=dent[:].bitcast(I32),
                    scalar1=-1, scalar2=RECIP_MAGIC, op0=MULT, op1=ADD,
                )
                r0 = r0i.bitcast(F32)
                td = lrp.tile([65, TQ], F32, tag="td")
                nc.vector.tensor_tensor(
                    out=td[:], in0=dent[:], in1=r0[:], op=MULT,
                )
                nc.vector.scalar_tensor_tensor(
                    out=lrall[:], in0=td[:], scalar=-2.0,
                    in1=r0[:], op0=ADD, op1=MULT,
                )
            psb0 = ps_misc.tile([128, TQ], F32, tag="misc", name="psb0")
            nc.tensor.matmul(
                psb0[:], lhsT=ones_sb[0:65, 0:128], rhs=lrall[:],
                start=True, stop=True,
            )
            with hp(HP_NORM):
                nc.vector.tensor_mul(ot[:, 0, :], otmp0[:], psb0[:])
            psb1 = ps_misc.tile([128, TQ], F32, tag="misc", name="psb1")
            nc.tensor.matmul(
                psb1[64:128, :], lhsT=ones_sb[0:1, 192:256], rhs=lrall[0:1, :],
                start=True, stop=True,
            )
            with hp(HP_NORM):
                nc.vector.tensor_mul(
                    ot[64:128, 1, :], otmp1[64:128, :], psb1[64:128, :]
                )
        norm_cb = norm_chunk

        # ---- output projection, software-pipelined by one chunk ----
        # proj(j-1) is emitted here so the PE has attn(j) work to cover the
        # normalization latency of chunk j-1.
        if j > 0:
            _proj(nc, ps_misc, stp, wp_sb, outT_r, prev_ot, j - 1, nc.gpsimd)
        prev_ot = ot
    norm_cb()
    _proj(nc, ps_misc, stp, wp_sb, outT_r, prev_ot, NJ - 1, nc.sync)


def build_nc():
    nc = bacc.Bacc("TRN2", target_bir_lowering=False, debug=False)
    t = {}
    t["xT"] = nc.dram_tensor("xT", [C, T], BF16, kind="ExternalInput").ap()
    t["wqk"] = nc.dram_tensor("wqk", [C, NQK], BF16, kind="ExternalInput").ap()
    t["wv"] = nc.dram_tensor("wv", [C, NV], BF16, kind="ExternalInput").ap()
    t["bqk"] = nc.dram_tensor("bqk", [3, 128], F32, kind="ExternalInput").ap()
    t["wproj"] = nc.dram_tensor("wproj", [NSLOT, 128, C], BF16, kind="ExternalInput").ap()
    t["masks"] = nc.dram_tensor("masks", [128, 1280], BF16, kind="ExternalInput").ap()
    t["outT"] = nc.dram_tensor("outT", [C, T], BF16, kind="ExternalOutput").ap()
    with tile.TileContext(nc) as tc:
        _mhsa_body(tc, t)
    nc.compile()
    return nc


def make_in_maps(x, W_qkv, b_qkv, W_proj):
    """Shard the full inputs into one input map per core."""
    x = np.asarray(x, dtype=np.float32)
    W_qkv = np.asarray(W_qkv, dtype=np.float32)
    b_qkv = np.asarray(b_qkv, dtype=np.float32)
    W_proj = np.asarray(W_proj, dtype=np.float32)

    q_idx = np.arange(TQ)
    p_idx = np.arange(128)
    m4 = np.zeros((4, 128, TQ), dtype=np.float32)
    for r in range(4):
        m4[r] = (p_idx[:, None] <= (q_idx[None, :] - 128 * r)).astype(np.float32)
    import ml_dtypes
    masks = np.concatenate(
        [m4[0], m4[1][:, 128:], m4[2][:, 256:], m4[3][:, 384:]], axis=1
    ).astype(ml_dtypes.bfloat16)  # [128, 512+384+256+128 = 1280]

    in_maps = []
    for c in range(NCORES):
        b = c // GPB
        g = c % GPB
        heads = [HPC * g + h for h in range(HPC)]

        def wslice(qty, h):
            off = {"Q": 0, "K": C, "V": 2 * C}[qty] + heads[h] * D
            return W_qkv[:, off:off + D]

        def bslice(qty, h):
            off = {"Q": 0, "K": C, "V": 2 * C}[qty] + heads[h] * D
            return b_qkv[off:off + D]

        # chunks: [Q0|Q1], [K0|K1], [Q2|K2]
        wqk = np.concatenate(
            [wslice("Q", 0), wslice("Q", 1),
             wslice("K", 0), wslice("K", 1),
             wslice("Q", 2), wslice("K", 2)], axis=1
        )
        bqk = np.zeros((3, 128), dtype=np.float32)
        bqk[0, 0:64] = bslice("Q", 0)
        bqk[0, 64:128] = bslice("Q", 1)
        bqk[1, 0:64] = bslice("K", 0)
        bqk[1, 64:128] = bslice("K", 1)
        bqk[2, 0:64] = bslice("Q", 2)
        bqk[2, 64:128] = bslice("K", 2)

        wv = np.concatenate([wslice("V", h) for h in range(HPC)], axis=1)

        # negated: the on-core normalize computes -O/den (see kernel comment)
        # slot0 = head2 rows 0:64 + head0 rows 64:128; slot1 = head1 rows 64:128
        wp = np.zeros((NSLOT, 128, C), dtype=np.float32)
        wp[0, 0:64] = -W_proj[heads[2] * D:(heads[2] + 1) * D, :]
        wp[0, 64:128] = -W_proj[heads[0] * D:(heads[0] + 1) * D, :]
        wp[1, 64:128] = -W_proj[heads[1] * D:(heads[1] + 1) * D, :]

        in_maps.append({
            "xT": np.ascontiguousarray(x[b].T).astype(ml_dtypes.bfloat16),
            "wqk": wqk.astype(ml_dtypes.bfloat16),
            "wv": wv.astype(ml_dtypes.bfloat16),
            "bqk": bqk,
            "wproj": wp.astype(ml_dtypes.bfloat16),
            "masks": masks,
        })
    return in_maps


def run_cores(inputs, trace=False, **kw):
    nc = build_nc()
    in_maps = make_in_maps(
        inputs["x"], inputs["W_qkv"], inputs["b_qkv"], inputs["W_proj"]
    )
    res = run_bass_kernel_spmd(nc, in_maps, list(range(NCORES)), trace=trace, **kw)
    return res


def gather(results, b_proj, b_qkv, W_proj):
    b_proj = np.asarray(b_proj, dtype=np.float32)
    b_qkv = np.asarray(b_qkv, dtype=np.float32)
    W_proj = np.asarray(W_proj, dtype=np.float32)
    # V bias commutes through softmax: out += b_v @ W_proj (host-side)
    b_eff = b_proj + b_qkv[2 * C:3 * C] @ W_proj
    out = np.zeros((B, T, C), dtype=np.float32)
    for c in range(NCORES):
        out[c // GPB] += results[c]["outT"].astype(np.float32).T
    out += b_eff
    return out


def kernel(x, W_qkv, b_qkv, W_proj, b_proj):
    res = run_cores(
        {"x": x, "W_qkv": W_qkv, "b_qkv": b_qkv, "W_proj": W_proj}
    )
    return gather(res.results, b_proj, b_qkv, W_proj)


# revision 41
# speedup vs baseline: 1.0193x; 1.0193x over previous
"""Multi-head causal self-attention (B=2, T=4096, C=768, H=12, D=64) on 8 NeuronCores.

Sharding: core c handles batch b = c // 4 and a group of 3 heads (c % 4).
Each core runs a fused flash-attention pipeline per 512-column tq chunk:
Q/K projection (3 weight chunks) -> V projection in natural [key, d] layout
via swapped matmuls (lhsT = x.T key-block) -> streaming softmax(QK^T)V ->
output projection, producing a partial (pre-bias) out.T [768, 4096] in bf16.
The host sums the 4 partials per batch and adds b_proj + b_v @ W_proj (the
V bias commutes through softmax normalization, so V is projected bias-free).

All score/attnV matmuls run in bf16 (scores are O(+-6); bf16 q/k adds ~0.3%
noise). Softmax skips max-subtraction; the denominator comes from an
appended ones-column in V. exp() is split across engines: diagonal (masked)
pairs use exact exp on the Activation engine with gpsimd mask-multiplies;
off-diagonal pairs alternate between Activation-exp and a Schraudolph
approximation on the Vector engine computed directly in int16
(int16(x*A/2^16 + B/2^16) is the top half of the fp32 exp bit pattern, i.e.
bf16 -- read back via a contiguous bitcast, full-rate for the PE).

The per-core QKV weight columns live in 3 chunks of 128: [Q0|Q1], [K0|K1],
[Q2|K2]. Q0/Q1 share qt slot 0 (zero-padded KT halves kill cross terms);
Q2 uses slot 1 (upper half zero). K2's projection lands on partitions
64:128 but must contract against Q2 on 0:64, so it is staged into KT[2]'s
(unused) upper rows and partition-shifted down by an SBUF->SBUF DMA --
engines cannot cross partition lanes, DMA can.

Normalization: -1/den per head via a bit-affine reciprocal seed plus one
Newton step on the Vector engine (sign absorbed by negating W_proj
host-side), written as fp16 rows of a shared tile; one ones-weighted fp16
matmul per ot slot broadcasts them across partitions.
"""

from contextlib import ExitStack

import numpy as np

import concourse.bass as bass
import concourse.tile as tile
from concourse import bacc
from concourse import mybir
from concourse._compat import with_exitstack
from concourse.bass_utils import run_bass_kernel_spmd

F32 = mybir.dt.float32
F16 = mybir.dt.float16
BF16 = mybir.dt.bfloat16
I16 = mybir.dt.int16
I32 = mybir.dt.int32
EXP = mybir.ActivationFunctionType.Exp
IDENT = mybir.ActivationFunctionType.Identity
MULT = mybir.AluOpType.mult
ADD = mybir.AluOpType.add

B, T, C = 2, 4096, 768
H, D = 12, 64
NCORES = 8
HPC = 3           # heads per core
GPB = NCORES // B  # head-group cores per batch (4)
TQ = 512          # tq chunk width
NJ = T // TQ      # 8
TKB = 128         # tk block
NB = T // TKB     # 32
KC = C // 128     # 6 contraction chunks for the projections
NQK = 3 * 128     # per-core Q/K weight columns
NV = HPC * D      # 192 per-core V weight columns
SCALE = 1.0 / np.sqrt(D)

# Schraudolph exp: exp(x) ~= bitcast_f32(int32(x * 2^23/ln2 + (127<<23) - CADJ))
# computed directly in the top 16 bits (== bf16) via an int16 tensor_scalar.
SCH_A = 12102203.0
SCH_CADJ = 366393.0
SCH_B = float((127 << 23)) - SCH_CADJ
RECIP_MAGIC = 0x7EF311C3  # bit-affine 1/x seed; one Newton step -> +-0.26%

import os
SCH_8THS = int(os.environ.get("SCH_8THS", "4"))  # ip%8 < this -> DVE exp
PIPE_ENABLE = os.environ.get("PIPE_ENABLE", "1") == "1"
# priority boosts (0=off). CAUTION: boosting an op whose dependency is
# produced late on another engine reorders it ahead of that engine's
# consumers and deadlocks the in-order streams.
HP_QKV = int(os.environ.get("HP_QKV", "0"))
HP_NORM = int(os.environ.get("HP_NORM", "0"))
WARMUP_MMS = int(os.environ.get("WARMUP_MMS", "80"))

# Vp block layout [128 keys, 128 cols]: per head, V data occupies VCOL..VCOL+64
# and a ones-column at ONES_COL supplies the softmax denominator. attnV output
# partitions = lhsT free index, so heads 0/1 land their O at rows 64:128 and
# head 2 at rows 0:64 -> heads pack into two 128-row ot slots for the output
# projection (slot0 = h2 rows 0:64 + h0 rows 64:128; slot1 = h1 rows 64:128).
VCOL = {0: 64, 1: 64, 2: 0}
ONES_COL = {0: 32, 1: 0, 2: 64}  # distinct + 32-aligned: den rows 32/0/64
LHS_W = {0: 128, 1: 128, 2: 65}  # attnV lhsT free width
DEN_ROW = {0: 32, 1: 0, 2: 64}  # pso row holding the denominator
O_ROW = {0: 64, 1: 64, 2: 0}    # pso/ot row base of the 64 output dims
OT_SLOT = {0: 0, 1: 1, 2: 0}    # ot slot per head
QT_SLOT = {0: 0, 1: 0, 2: 1}    # qt slot per head
NSLOT = 2


def _proj(nc, ps_misc, stp, wp_sb, outT_r, ot, j, qeng):
    jsl = slice(j * TQ, (j + 1) * TQ)
    for m in range(KC):
        ps3 = ps_misc.tile([128, TQ], F32, tag="misc", name="ps3")
        for sl in range(NSLOT):
            nc.tensor.matmul(
                ps3[:],
                lhsT=wp_sb[:, sl, m * 128:(m + 1) * 128],
                rhs=ot[:, sl, :],
                start=(sl == 0),
                stop=(sl == NSLOT - 1),
            )
        st = stp.tile([128, TQ], BF16, tag="st", name="st")
        if m % 2 == 0:
            nc.vector.tensor_copy(st[:], ps3[:])
        else:
            nc.scalar.copy(st[:], ps3[:])
        # mid-kernel stores ride the gpsimd DGE queue so their triggers
        # (waiting on st copies) never head-of-line block the sync queue's
        # xt prefetches; the final chunk uses sync (prompt completion).
        qeng.dma_start(outT_r[:, m, jsl], st[:])


@with_exitstack
def _mhsa_body(ctx: ExitStack, tc: tile.TileContext, t):
    from contextlib import nullcontext

    nc = tc.nc

    def hp(offset):
        return tc.high_priority(offset) if offset else nullcontext()

    norm_cb = None  # deferred per-chunk normalize (flushed next chunk)
    xT_r = t["xT"].rearrange("(kc p) t -> p kc t", p=128)
    outT_r = t["outT"].rearrange("(mo p) t -> p mo t", p=128)
    wqk_r = t["wqk"].rearrange("(kc p) m -> p kc m", p=128)
    wv_r = t["wv"].rearrange("(kc p) m -> p kc m", p=128)

    const = ctx.enter_context(tc.tile_pool(name="const", bufs=1))
    persist = ctx.enter_context(tc.tile_pool(name="persist", bufs=1))
    xpool = ctx.enter_context(tc.tile_pool(name="xpool", bufs=2))
    ptp = ctx.enter_context(tc.tile_pool(name="ptp", bufs=4))
    ptip = ctx.enter_context(tc.tile_pool(name="ptip", bufs=4))
    stp = ctx.enter_context(tc.tile_pool(name="stp", bufs=3))
    lrp = ctx.enter_context(tc.tile_pool(name="lrp", bufs=4))
    otmpp = ctx.enter_context(tc.tile_pool(name="otmpp", bufs=4))

    ps_pair = ctx.enter_context(tc.tile_pool(name="ps_pair", bufs=2, space="PSUM"))
    ps_o = ctx.enter_context(tc.tile_pool(name="ps_o", bufs=2, space="PSUM"))
    ps_misc = ctx.enter_context(tc.tile_pool(name="ps_misc", bufs=2, space="PSUM"))

    # PE warmup: dummy matmuls during the initial input-DMA wait keep the
    # HAM activity monitor busy so the clock gate opens (1.2 -> 2.4 GHz)
    # just as the first real matmul lands (~3.4us of sustained activity).
    # Emitted first (vector memset, not gpsimd: the big Vp/qt memsets would
    # delay it) so the PE stream starts immediately.
    if WARMUP_MMS:
        wu = const.tile([128, 256], BF16)
        nc.vector.memset(wu[:], 0.0)
        wups = ps_misc.tile([128, TQ], F32, tag="misc", name="wups")
        for _ in range(WARMUP_MMS):
            nc.tensor.matmul(
                wups[:, 0:256], lhsT=wu[:, 0:128], rhs=wu[:, 0:256],
                start=True, stop=True,
            )

    # chunk 0's x slices go first: the first Q/K matmul needs xt(kc0)+wqk(kc0)
    xt0 = xpool.tile([128, KC, TQ], BF16, tag="xt")
    wqk_sb = const.tile([128, KC, NQK], BF16)
    wv_sb = const.tile([128, KC, NV], BF16)
    for kc in range(KC):
        nc.sync.dma_start(xt0[:, kc, :], xT_r[:, kc, 0:TQ])
        nc.sync.dma_start(wqk_sb[:, kc, :], wqk_r[:, kc, :])
    bias_sb = const.tile([128, 3], F32)
    nc.sync.dma_start(bias_sb[:], t["bqk"].rearrange("m p -> p m"))
    for kc in range(KC):  # V weights: needed only mid-chunk, loaded after
        nc.sync.dma_start(wv_sb[:, kc, :], wv_r[:, kc, :])
    mask_sb = const.tile([128, 1280], BF16)
    nc.scalar.dma_start(mask_sb[:], t["masks"])
    # ones_sb: fp16 lhsT patterns for the -1/den broadcast matmuls.
    # cols 0:128 = slot0 (row 32 -> out 64:128 (h0), row 64 -> out 0:64 (h2)),
    # cols 128:256 = slot1 (row 0 -> out 64:128 (h1)).
    ones_sb = const.tile([128, 256], F16)
    nc.gpsimd.memset(ones_sb[:], 0.0)
    nc.gpsimd.memset(ones_sb[32:33, 64:128], 1.0)
    nc.gpsimd.memset(ones_sb[64:65, 0:64], 1.0)
    nc.gpsimd.memset(ones_sb[0:1, 192:256], 1.0)
    wp_sb = const.tile([128, NSLOT, C], BF16)
    nc.scalar.dma_start(wp_sb[:], t["wproj"].rearrange("h p m -> p h m"))

    KT = [persist.tile([128, T], BF16, tag=f"KT{h}", name=f"KT{h}") for h in range(HPC)]
    # V in natural [key, d] layout: h0/h1 share a merged tile (same VCOL) so
    # one DVE copy per key-block covers both; h2 has its own (VCOL 0).
    Vp01 = persist.tile([128, NB, 2, 128], BF16, tag="Vp01", name="Vp01")
    Vp2 = persist.tile([128, NB, 128], BF16, tag="Vp2", name="Vp2")
    qts = [
        persist.tile([128, NSLOT, TQ], BF16, tag=f"qt{i}", name=f"qt{i}")
        for i in range(2)
    ]
    ots = [
        persist.tile([128, NSLOT, TQ], BF16, tag=f"ot{i}", name=f"ot{i}")
        for i in range(2)
    ]
    # den staging rows (h1=0, h0=32, h2=64) for one batched reciprocal chain
    # per chunk; filler rows hold 1.0 so the chain stays finite on them.
    dent = persist.tile([65, TQ], F32, tag="dent", name="dent")
    lrall = persist.tile([65, TQ], F16, tag="lrall", name="lrall")

    # (KT[0]/KT[1] pad halves need no zeroing: the row-tiled score matmuls
    # contract only each head's 64 data rows)
    nc.gpsimd.memset(Vp01[:, :, :, 0:64], 0.0)
    nc.gpsimd.memset(Vp01[:, :, 0, 32:33], 1.0)
    nc.gpsimd.memset(Vp01[:, :, 1, 0:1], 1.0)
    nc.gpsimd.memset(Vp2[:, :, 64:65], 1.0)
    nc.gpsimd.memset(dent[:], 1.0)
    for i in range(2):
        nc.gpsimd.memset(qts[i][64:128, 1, :], 0.0)
        nc.gpsimd.memset(ots[i][0:64, 1, :], 0.0)



    for j in range(NJ):
        jsl = slice(j * TQ, (j + 1) * TQ)

        # ---- Q/K/V projection for this tq chunk ----
        if j == 0:
            xt = xt0
        else:
            xt = xpool.tile([128, KC, TQ], BF16, tag="xt")
            for kc in range(KC):
                nc.sync.dma_start(xt[:, kc, :], xT_r[:, kc, jsl])
        qt = qts[j % 2]

        # W-chunk order: [Q2|K2] first (K2's shift DMA latency), then
        # [Q0|Q1] (qt slot0 gates h0/h1 score starts), V key-blocks, then
        # [K0|K1] last (K blocks gate only each head's diagonal pairs).
        for m in (2, 0):
            ps = ps_misc.tile([128, TQ], F32, tag="misc", name="psqk")
            for kc in range(KC):
                nc.tensor.matmul(
                    ps[:],
                    lhsT=wqk_sb[:, kc, m * 128:(m + 1) * 128],
                    rhs=xt[:, kc, :],
                    start=(kc == 0),
                    stop=(kc == KC - 1),
                )
            with hp(HP_QKV):
                if m == 0:  # [Q0|Q1] -> qt slot0, one full-height biased copy
                    nc.scalar.activation(
                        out=qt[:, 0, :], in_=ps[:], func=IDENT,
                        bias=bias_sb[:, 0:1],
                    )
                else:  # [Q2|K2]
                    nc.vector.tensor_scalar(
                        out=qt[0:64, 1, :], in0=ps[0:64, :],
                        scalar1=bias_sb[0:64, 2:3], scalar2=None, op0=ADD,
                    )
                    # K2: stage into KT[2] rows 64:128 (qt slot1's zero upper
                    # half makes them don't-care for scores), then partition-
                    # shift down with an SBUF->SBUF DMA.
                    nc.scalar.activation(
                        out=KT[2][64:128, jsl], in_=ps[64:128, :], func=IDENT,
                        bias=bias_sb[64:128, 2:3],
                    )
                    nc.gpsimd.dma_start(KT[2][0:64, jsl], KT[2][64:128, jsl])

        for s in range(4):  # V projection, natural [key, d] layout
            psv = ps_misc.tile([128, TQ], F32, tag="misc", name="psv")
            for kc in range(KC):
                nc.tensor.matmul(
                    psv[:, 0:NV],
                    lhsT=xt[:, kc, s * 128:(s + 1) * 128],
                    rhs=wv_sb[:, kc, :],
                    start=(kc == 0),
                    stop=(kc == KC - 1),
                )
            with hp(HP_QKV):
                nc.vector.tensor_copy(
                    out=Vp01[:, 4 * j + s, :, 64:128], in_=psv[:, 0:128]
                )
                nc.vector.tensor_copy(
                    out=Vp2[:, 4 * j + s, 0:64], in_=psv[:, 128:192]
                )

        ps = ps_misc.tile([128, TQ], F32, tag="misc", name="psk")
        for kc in range(KC):
            nc.tensor.matmul(
                ps[:],
                lhsT=wqk_sb[:, kc, 128:256],
                rhs=xt[:, kc, :],
                start=(kc == 0),
                stop=(kc == KC - 1),
            )
        with hp(HP_QKV):
            nc.vector.tensor_scalar(
                out=KT[0][0:64, jsl], in0=ps[0:64, :],
                scalar1=bias_sb[0:64, 1:2], scalar2=None, op0=ADD,
            )
            nc.scalar.activation(
                out=KT[1][64:128, jsl], in_=ps[64:128, :], func=IDENT,
                bias=bias_sb[64:128, 1:2],
            )

        # ---- streaming attention for this tq chunk ----
        ot = ots[j % 2]
        otmp0 = otmpp.tile([128, TQ], F32, tag="otmp", name="otmp0")
        otmp1 = otmpp.tile([128, TQ], F32, tag="otmp", name="otmp1")
        nblk = 4 * j + 4
        npair = nblk // 2
        def pair_geom(ip):
            i0, i1 = 2 * ip, 2 * ip + 1
            # column trim offsets: block i only contributes to tq columns
            # >= 128*(i-4j) within this chunk
            offs = [max(0, 128 * (i - 4 * j)) for i in (i0, i1)]
            ns = [TQ - o for o in offs]
            return (i0, i1), offs, ns, [0, ns[0]], ns[0] + ns[1]

        def exp_act(pss, w):
            pt = ptp.tile([128, 2 * TQ], BF16, tag="pt")
            nc.scalar.activation(
                out=pt[:, :w], in_=pss[:, :w], func=EXP, scale=SCALE
            )
            return pt

        def exp_split(pss):
            # off-diag pair (two full 512 blocks): block0 exact exp on ACT
            # concurrently with block1 Schraudolph on DVE -- halves the exp
            # latency per pair so scores(ip+1) never starve the PE.
            pt = ptp.tile([128, 2 * TQ], BF16, tag="pt")
            nc.scalar.activation(
                out=pt[:, 0:TQ], in_=pss[:, 0:TQ], func=EXP, scale=SCALE
            )
            pt16 = ptip.tile([128, 2 * TQ], I16, tag="pt16")
            nc.vector.tensor_scalar(
                out=pt16[:, TQ:2 * TQ], in0=pss[:, TQ:2 * TQ],
                scalar1=SCH_A * SCALE / 65536.0, scalar2=SCH_B / 65536.0,
                op0=MULT, op1=ADD,
            )
            ptb = pt16.bitcast(BF16)
            return lambda s, n: (pt[:, s:s + n] if s < TQ
                                 else ptb[:, s:s + n])

        def exp_diag(pss, w):
            pt = exp_act(pss, w)
            moff = 0 if w == 896 else 896
            nc.vector.tensor_mul(
                pt[:, :w], pt[:, :w], mask_sb[:, moff:moff + w]
            )
            return lambda s, n: pt[:, s:s + n]

        # ---- head 2 alone (qt slot1; upper contraction half is zero) ----
        # software pipeline by one pair: emit scores(ip) one pair ahead of
        # attnV(ip) so the PE always has exp-independent work queued.
        pso2 = ps_o.tile([128, TQ], F32, tag="pso", name="pso2")
        pending = None  # (rhs, offs, ns, starts, (i0, i1))
        for ip in range(npair + 1):
            if ip < npair:
                blks, offs, ns, starts, w = pair_geom(ip)
                pss = ps_pair.tile([128, 2 * TQ], F32, tag="pss")
                for n, i in enumerate(blks):
                    nc.tensor.matmul(
                        pss[:, starts[n]:starts[n] + ns[n]],
                        lhsT=KT[2][:, i * 128:(i + 1) * 128],
                        rhs=qt[:, 1, offs[n]:TQ],
                        start=True,
                        stop=True,
                    )
                if ip - 2 * j >= 0:
                    rhs_of = exp_diag(pss, w)
                else:
                    rhs_of = exp_split(pss)
                cur = (rhs_of, offs, ns, starts, blks)
            else:
                cur = None
            if not PIPE_ENABLE:
                pending, cur = cur, None
            if pending is not None:
                (p_rhs, p_offs, p_ns, p_starts, p_blks) = pending
                for n, i in enumerate(p_blks):
                    nc.tensor.matmul(
                        pso2[0:65, p_offs[n]:TQ],
                        lhsT=Vp2[:, i, 0:65],
                        rhs=p_rhs(p_starts[n], p_ns[n]),
                        start=(i == 0),
                        stop=(i == nblk - 1),
                    )
            pending = cur

        if norm_cb is not None:
            # previous chunk's deferred normalize: flushed after h2's pair
            # emission (which covers the chain latency) and before h2's
            # dent write (the chain reads all dent rows).
            norm_cb()
            norm_cb = None
        nc.vector.tensor_copy(otmp0[0:64, :], pso2[0:64, :])
        nc.scalar.copy(dent[64:65, :], pso2[64:65, :])

        # ---- heads 0 and 1 jointly: their K/Q data live in complementary
        # 64-partition halves, so each score pair runs as two concurrent
        # 64-deep row-tiled matmuls (tile rows 0:64 for h0, 64:128 for h1).
        pso0 = ps_o.tile([128, TQ], F32, tag="pso", name="pso0")
        pso1 = ps_o.tile([128, TQ], F32, tag="pso", name="pso1")
        pending = None
        for ip in range(npair + 1):
            if ip < npair:
                blks, offs, ns, starts, w = pair_geom(ip)
                pss0 = ps_pair.tile([128, 2 * TQ], F32, tag="pss", name="pss0")
                pss1 = ps_pair.tile([128, 2 * TQ], F32, tag="pss", name="pss1")
                for n, i in enumerate(blks):
                    nc.tensor.matmul(
                        pss0[:, starts[n]:starts[n] + ns[n]],
                        lhsT=KT[0][0:64, i * 128:(i + 1) * 128],
                        rhs=qt[0:64, 0, offs[n]:TQ],
                        start=True,
                        stop=True,
                    )
                    nc.tensor.matmul(
                        pss1[:, starts[n]:starts[n] + ns[n]],
                        lhsT=KT[1][64:128, i * 128:(i + 1) * 128],
                        rhs=qt[64:128, 0, offs[n]:TQ],
                        start=True,
                        stop=True,
                    )
                if ip - 2 * j >= 0:  # diagonal: exact exp + masks, both heads
                    r0f = exp_diag(pss0, w)
                    r1f = exp_diag(pss1, w)
                else:
                    r0f = exp_split(pss0)
                    r1f = exp_split(pss1)
                cur = ((r0f, r1f), offs, ns, starts, blks)
            else:
                cur = None
            if not PIPE_ENABLE:
                pending, cur = cur, None
            if pending is not None:
                ((p0, p1), p_offs, p_ns, p_starts, p_blks) = pending
                for n, i in enumerate(p_blks):
                    nc.tensor.matmul(
                        pso0[:, p_offs[n]:TQ],
                        lhsT=Vp01[:, i, 0, :],
                        rhs=p0(p_starts[n], p_ns[n]),
                        start=(i == 0),
                        stop=(i == nblk - 1),
                    )
                    nc.tensor.matmul(
                        pso1[:, p_offs[n]:TQ],
                        lhsT=Vp01[:, i, 1, :],
                        rhs=p1(p_starts[n], p_ns[n]),
                        start=(i == 0),
                        stop=(i == nblk - 1),
                    )
            pending = cur

        # copy O rows to SBUF (releases the PSUM banks) and stage den rows
        # into the shared dent tile (h1->0, h0->32, h2->64); the reciprocal
        # chain runs once per chunk, batched.
        nc.vector.tensor_copy(otmp0[64:128, :], pso0[64:128, :])
        nc.scalar.copy(dent[32:33, :], pso0[32:33, :])
        nc.vector.tensor_copy(otmp1[64:128, :], pso1[64:128, :])
        nc.vector.tensor_copy(dent[0:1, :], pso1[0:1, :])

        # deferred normalize for this chunk: one batched reciprocal chain
        # -1/den over dent rows {0,32,64} (filler rows hold 1.0), fp16
        # ones-weighted broadcast matmuls per ot slot, then the final muls.
        def norm_chunk(ot=ot, otmp0=otmp0, otmp1=otmp1):
            with hp(HP_NORM):
                r0i = lrp.tile([65, TQ], I32, tag="r0i")
                nc.vector.tensor_scalar(
                    out=r0i[:], in0=dent[:].bitcast(I32),
                    scalar1=-1, scalar2=RECIP_MAGIC, op0=MULT, op1=ADD,
                )
                r0 = r0i.bitcast(F32)
                td = lrp.tile([65, TQ], F32, tag="td")
                nc.vector.tensor_tensor(
                    out=td[:], in0=dent[:], in1=r0[:], op=MULT,
                )
                nc.vector.scalar_tensor_tensor(
                    out=lrall[:], in0=td[:], scalar=-2.0,
                    in1=r0[:], op0=ADD, op1=MULT,
                )
            psb0 = ps_misc.tile([128, TQ], F32, tag="misc", name="psb0")
            nc.tensor.matmul(
                psb0[:], lhsT=ones_sb[0:65, 0:128], rhs=lrall[:],
                start=True, stop=True,
            )
            with hp(HP_NORM):
                nc.vector.tensor_mul(ot[:, 0, :], otmp0[:], psb0[:])
            psb1 = ps_misc.tile([128, TQ], F32, tag="misc", name="psb1")
            nc.tensor.matmul(
                psb1[64:128, :], lhsT=ones_sb[0:1, 192:256], rhs=lrall[0:1, :],
                start=True, stop=True,
            )
            with hp(HP_NORM):
                nc.vector.tensor_mul(
                    ot[64:128, 1, :], otmp1[64:128, :], psb1[64:128, :]
                )
        norm_cb = norm_chunk

        # ---- output projection, software-pipelined by one chunk ----
        # proj(j-1) is emitted here so the PE has attn(j) work to cover the
        # normalization latency of chunk j-1.
        if j > 0:
            _proj(nc, ps_misc, stp, wp_sb, outT_r, prev_ot, j - 1, nc.gpsimd)
        prev_ot = ot
    norm_cb()
    _proj(nc, ps_misc, stp, wp_sb, outT_r, prev_ot, NJ - 1, nc.sync)


def build_nc():
    nc = bacc.Bacc("TRN2", target_bir_lowering=False, debug=False)
    t = {}
    t["xT"] = nc.dram_tensor("xT", [C, T], BF16, kind="ExternalInput").ap()
    t["wqk"] = nc.dram_tensor("wqk", [C, NQK], BF16, kind="ExternalInput").ap()
    t["wv"] = nc.dram_tensor("wv", [C, NV], BF16, kind="ExternalInput").ap()
    t["bqk"] = nc.dram_tensor("bqk", [3, 128], F32, kind="ExternalInput").ap()
    t["wproj"] = nc.dram_tensor("wproj", [NSLOT, 128, C], BF16, kind="ExternalInput").ap()
    t["masks"] = nc.dram_tensor("masks", [128, 1280], BF16, kind="ExternalInput").ap()
    t["outT"] = nc.dram_tensor("outT", [C, T], BF16, kind="ExternalOutput").ap()
    with tile.TileContext(nc) as tc:
        _mhsa_body(tc, t)
    nc.compile()
    return nc


def make_in_maps(x, W_qkv, b_qkv, W_proj):
    """Shard the full inputs into one input map per core."""
    x = np.asarray(x, dtype=np.float32)
    W_qkv = np.asarray(W_qkv, dtype=np.float32)
    b_qkv = np.asarray(b_qkv, dtype=np.float32)
    W_proj = np.asarray(W_proj, dtype=np.float32)

    q_idx = np.arange(TQ)
    p_idx = np.arange(128)
    m4 = np.zeros((4, 128, TQ), dtype=np.float32)
    for r in range(4):
        m4[r] = (p_idx[:, None] <= (q_idx[None, :] - 128 * r)).astype(np.float32)
    import ml_dtypes
    masks = np.concatenate(
        [m4[0], m4[1][:, 128:], m4[2][:, 256:], m4[3][:, 384:]], axis=1
    ).astype(ml_dtypes.bfloat16)  # [128, 512+384+256+128 = 1280]

    in_maps = []
    for c in range(NCORES):
        b = c // GPB
        g = c % GPB
        heads = [HPC * g + h for h in range(HPC)]

        def wslice(qty, h):
            off = {"Q": 0, "K": C, "V": 2 * C}[qty] + heads[h] * D
            return W_qkv[:, off:off + D]

        def bslice(qty, h):
            off = {"Q": 0, "K": C, "V": 2 * C}[qty] + heads[h] * D
            return b_qkv[off:off + D]

        # chunks: [Q0|Q1], [K0|K1], [Q2|K2]
        wqk = np.concatenate(
            [wslice("Q", 0), wslice("Q", 1),
             wslice("K", 0), wslice("K", 1),
             wslice("Q", 2), wslice("K", 2)], axis=1
        )
        bqk = np.zeros((3, 128), dtype=np.float32)
        bqk[0, 0:64] = bslice("Q", 0)
        bqk[0, 64:128] = bslice("Q", 1)
        bqk[1, 0:64] = bslice("K", 0)
        bqk[1, 64:128] = bslice("K", 1)
        bqk[2, 0:64] = bslice("Q", 2)
        bqk[2, 64:128] = bslice("K", 2)

        wv = np.concatenate([wslice("V", h) for h in range(HPC)], axis=1)

        # negated: the on-core normalize computes -O/den (see kernel comment)
        # slot0 = head2 rows 0:64 + head0 rows 64:128; slot1 = head1 rows 64:128
        wp = np.zeros((NSLOT, 128, C), dtype=np.float32)
        wp[0, 0:64] = -W_proj[heads[2] * D:(heads[2] + 1) * D, :]
        wp[0, 64:128] = -W_proj[heads[0] * D:(heads[0] + 1) * D, :]
        wp[1, 64:128] = -W_proj[heads[1] * D:(heads[1] + 1) * D, :]

        in_maps.append({
            "xT": np.ascontiguousarray(x[b].T).astype(ml_dtypes.bfloat16),
            "wqk": wqk.astype(ml_dtypes.bfloat16),
            "wv": wv.astype(ml_dtypes.bfloat16),
            "bqk": bqk,
            "wproj": wp.astype(ml_dtypes.bfloat16),
            "masks": masks,
        })
    return in_maps


def run_cores(inputs, trace=False, **kw):
    nc = build_nc()
    in_maps = make_in_maps(
        inputs["x"], inputs["W_qkv"], inputs["b_qkv"], inputs["W_proj"]
    )
    res = run_bass_kernel_spmd(nc, in_maps, list(range(NCORES)), trace=trace, **kw)
    return res


def gather(results, b_proj, b_qkv, W_proj):
    b_proj = np.asarray(b_proj, dtype=np.float32)
    b_qkv = np.asarray(b_qkv, dtype=np.float32)
    W_proj = np.asarray(W_proj, dtype=np.float32)
    # V bias commutes through softmax: out += b_v @ W_proj (host-side)
    b_eff = b_proj + b_qkv[2 * C:3 * C] @ W_proj
    out = np.zeros((B, T, C), dtype=np.float32)
    for c in range(NCORES):
        out[c // GPB] += results[c]["outT"].astype(np.float32).T
    out += b_eff
    return out


def kernel(x, W_qkv, b_qkv, W_proj, b_proj):
    res = run_cores(
        {"x": x, "W_qkv": W_qkv, "b_qkv": b_qkv, "W_proj": W_proj}
    )
    return gather(res.results, b_proj, b_qkv, W_proj)


# revision 45
# speedup vs baseline: 1.0755x; 1.0551x over previous
"""Multi-head causal self-attention (B=2, T=4096, C=768, H=12, D=64) on 8 NeuronCores.

Sharding: core c handles batch b = c // 4 and a group of 3 heads (c % 4).
Each core runs a fused flash-attention pipeline per 512-column tq chunk:
Q/K projection (3 weight chunks) -> V projection in natural [key, d] layout
via swapped matmuls (lhsT = x.T key-block) -> streaming softmax(QK^T)V ->
output projection, producing a partial (pre-bias) out.T [768, 4096] in bf16.
The host sums the 4 partials per batch and adds b_proj + b_v @ W_proj (the
V bias commutes through softmax normalization, so V is projected bias-free).

All score/attnV matmuls run in bf16 (scores are O(+-6); bf16 q/k adds ~0.3%
noise). Softmax skips max-subtraction; the denominator comes from an
appended ones-column in V. exp() is split across engines: diagonal (masked)
pairs use exact exp on the Activation engine with gpsimd mask-multiplies;
off-diagonal pairs alternate between Activation-exp and a Schraudolph
approximation on the Vector engine computed directly in int16
(int16(x*A/2^16 + B/2^16) is the top half of the fp32 exp bit pattern, i.e.
bf16 -- read back via a contiguous bitcast, full-rate for the PE).

The per-core QKV weight columns live in 3 chunks of 128: [Q0|Q1], [K0|K1],
[Q2|K2]. Q0/Q1 share qt slot 0 (zero-padded KT halves kill cross terms);
Q2 uses slot 1 (upper half zero). K2's projection lands on partitions
64:128 but must contract against Q2 on 0:64, so it is staged into KT[2]'s
(unused) upper rows and partition-shifted down by an SBUF->SBUF DMA --
engines cannot cross partition lanes, DMA can.

Normalization: -1/den per head via a bit-affine reciprocal seed plus one
Newton step on the Vector engine (sign absorbed by negating W_proj
host-side), written as fp16 rows of a shared tile; one ones-weighted fp16
matmul per ot slot broadcasts them across partitions.
"""

from contextlib import ExitStack

import numpy as np

import concourse.bass as bass
import concourse.tile as tile
from concourse import bacc
from concourse import mybir
from concourse._compat import with_exitstack
from concourse.bass_utils import run_bass_kernel_spmd

F32 = mybir.dt.float32
F16 = mybir.dt.float16
BF16 = mybir.dt.bfloat16
I16 = mybir.dt.int16
I32 = mybir.dt.int32
EXP = mybir.ActivationFunctionType.Exp
IDENT = mybir.ActivationFunctionType.Identity
MULT = mybir.AluOpType.mult
ADD = mybir.AluOpType.add

B, T, C = 2, 4096, 768
H, D = 12, 64
NCORES = 8
HPC = 3           # heads per core
GPB = NCORES // B  # head-group cores per batch (4)
TQ = 512          # tq chunk width
NJ = T // TQ      # 8
TKB = 128         # tk block
NB = T // TKB     # 32
KC = C // 128     # 6 contraction chunks for the projections
NQK = 3 * 128     # per-core Q/K weight columns
NV = HPC * D      # 192 per-core V weight columns
SCALE = 1.0 / np.sqrt(D)

# Schraudolph exp: exp(x) ~= bitcast_f32(int32(x * 2^23/ln2 + (127<<23) - CADJ))
# computed directly in the top 16 bits (== bf16) via an int16 tensor_scalar.
SCH_A = 12102203.0
SCH_CADJ = 366393.0
SCH_B = float((127 << 23)) - SCH_CADJ
RECIP_MAGIC = 0x7EF311C3  # bit-affine 1/x seed; one Newton step -> +-0.26%

import os
SCH_8THS = int(os.environ.get("SCH_8THS", "4"))  # ip%8 < this -> DVE exp
PIPE_ENABLE = os.environ.get("PIPE_ENABLE", "1") == "1"
# priority boosts (0=off). CAUTION: boosting an op whose dependency is
# produced late on another engine reorders it ahead of that engine's
# consumers and deadlocks the in-order streams.
HP_QKV = int(os.environ.get("HP_QKV", "0"))
HP_NORM = int(os.environ.get("HP_NORM", "0"))
WARMUP_MMS = int(os.environ.get("WARMUP_MMS", "80"))

# Vp block layout [128 keys, 128 cols]: per head, V data occupies VCOL..VCOL+64
# and a ones-column at ONES_COL supplies the softmax denominator. attnV output
# partitions = lhsT free index, so heads 0/1 land their O at rows 64:128 and
# head 2 at rows 0:64 -> heads pack into two 128-row ot slots for the output
# projection (slot0 = h2 rows 0:64 + h0 rows 64:128; slot1 = h1 rows 64:128).
VCOL = {0: 64, 1: 64, 2: 0}
ONES_COL = {0: 32, 1: 0, 2: 64}  # distinct + 32-aligned: den rows 32/0/64
LHS_W = {0: 128, 1: 128, 2: 65}  # attnV lhsT free width
DEN_ROW = {0: 32, 1: 0, 2: 64}  # pso row holding the denominator
O_ROW = {0: 64, 1: 64, 2: 0}    # pso/ot row base of the 64 output dims
OT_SLOT = {0: 0, 1: 1, 2: 0}    # ot slot per head
QT_SLOT = {0: 0, 1: 0, 2: 1}    # qt slot per head
NSLOT = 2


def _proj(nc, ps_misc, stp, wp_sb, outT_r, ot, j, qeng):
    jsl = slice(j * TQ, (j + 1) * TQ)
    for m in range(KC):
        ps3 = ps_misc.tile([128, TQ], F32, tag="misc", name="ps3")
        for sl in range(NSLOT):
            nc.tensor.matmul(
                ps3[:],
                lhsT=wp_sb[:, sl, m * 128:(m + 1) * 128],
                rhs=ot[:, sl, :],
                start=(sl == 0),
                stop=(sl == NSLOT - 1),
            )
        st = stp.tile([128, TQ], BF16, tag="st", name="st")
        if m % 2 == 0:
            nc.vector.tensor_copy(st[:], ps3[:])
        else:
            nc.scalar.copy(st[:], ps3[:])
        # mid-kernel stores ride the gpsimd DGE queue so their triggers
        # (waiting on st copies) never head-of-line block the sync queue's
        # xt prefetches; the final chunk uses sync (prompt completion).
        qeng.dma_start(outT_r[:, m, jsl], st[:])


@with_exitstack
def _mhsa_body(ctx: ExitStack, tc: tile.TileContext, t):
    from contextlib import nullcontext

    nc = tc.nc

    def hp(offset):
        return tc.high_priority(offset) if offset else nullcontext()

    norm_cb = None  # deferred per-chunk normalize (flushed next chunk)
    xT_r = t["xT"].rearrange("(kc p) t -> p kc t", p=128)
    outT_r = t["outT"].rearrange("(mo p) t -> p mo t", p=128)
    wqk_r = t["wqk"].rearrange("(kc p) m -> p kc m", p=128)
    wv_r = t["wv"].rearrange("(kc p) m -> p kc m", p=128)

    const = ctx.enter_context(tc.tile_pool(name="const", bufs=1))
    persist = ctx.enter_context(tc.tile_pool(name="persist", bufs=1))
    xpool = ctx.enter_context(tc.tile_pool(name="xpool", bufs=2))
    ptp = ctx.enter_context(tc.tile_pool(name="ptp", bufs=4))
    ptip = ctx.enter_context(tc.tile_pool(name="ptip", bufs=4))
    stp = ctx.enter_context(tc.tile_pool(name="stp", bufs=3))
    lrp = ctx.enter_context(tc.tile_pool(name="lrp", bufs=4))
    otmpp = ctx.enter_context(tc.tile_pool(name="otmpp", bufs=4))

    ps_pair = ctx.enter_context(tc.tile_pool(name="ps_pair", bufs=2, space="PSUM"))
    ps_o = ctx.enter_context(tc.tile_pool(name="ps_o", bufs=2, space="PSUM"))
    ps_misc = ctx.enter_context(tc.tile_pool(name="ps_misc", bufs=2, space="PSUM"))

    # PE warmup: dummy matmuls during the initial input-DMA wait keep the
    # HAM activity monitor busy so the clock gate opens (1.2 -> 2.4 GHz)
    # just as the first real matmul lands (~3.4us of sustained activity).
    # Emitted first (vector memset, not gpsimd: the big Vp/qt memsets would
    # delay it) so the PE stream starts immediately.
    if WARMUP_MMS:
        wu = const.tile([128, 256], BF16)
        nc.vector.memset(wu[:], 0.0)
        wups = ps_misc.tile([128, TQ], F32, tag="misc", name="wups")
        for _ in range(WARMUP_MMS):
            nc.tensor.matmul(
                wups[:, 0:256], lhsT=wu[:, 0:128], rhs=wu[:, 0:256],
                start=True, stop=True,
            )

    # chunk 0's x slices go first: the first Q/K matmul needs xt(kc0)+wqk(kc0)
    xt0 = xpool.tile([128, KC, TQ], BF16, tag="xt")
    wqk_sb = const.tile([128, KC, NQK], BF16)
    wv_sb = const.tile([128, KC, NV], BF16)
    for kc in range(KC):
        nc.sync.dma_start(xt0[:, kc, :], xT_r[:, kc, 0:TQ])
        nc.sync.dma_start(wqk_sb[:, kc, :], wqk_r[:, kc, :])
    bias_sb = const.tile([128, 3], F32)
    nc.sync.dma_start(bias_sb[:], t["bqk"].rearrange("m p -> p m"))
    for kc in range(KC):  # V weights: needed only mid-chunk, loaded after
        nc.sync.dma_start(wv_sb[:, kc, :], wv_r[:, kc, :])
    mask_sb = const.tile([128, 1280], BF16)
    nc.scalar.dma_start(mask_sb[:], t["masks"])
    # ones_sb: fp16 lhsT patterns for the -1/den broadcast matmuls.
    # cols 0:128 = slot0 (row 32 -> out 64:128 (h0), row 64 -> out 0:64 (h2)),
    # cols 128:256 = slot1 (row 0 -> out 64:128 (h1)).
    ones_sb = const.tile([128, 256], F16)
    nc.gpsimd.memset(ones_sb[:], 0.0)
    nc.gpsimd.memset(ones_sb[32:33, 64:128], 1.0)
    nc.gpsimd.memset(ones_sb[64:65, 0:64], 1.0)
    nc.gpsimd.memset(ones_sb[0:1, 192:256], 1.0)
    wp_sb = const.tile([128, NSLOT, C], BF16)
    nc.scalar.dma_start(wp_sb[:], t["wproj"].rearrange("h p m -> p h m"))

    KT = [persist.tile([128, T], BF16, tag=f"KT{h}", name=f"KT{h}") for h in range(HPC)]
    # V in natural [key, d] layout: h0/h1 share a merged tile (same VCOL) so
    # one DVE copy per key-block covers both; h2 has its own (VCOL 0).
    Vp01 = persist.tile([128, NB, 2, 128], BF16, tag="Vp01", name="Vp01")
    Vp2 = persist.tile([128, NB, 128], BF16, tag="Vp2", name="Vp2")
    qts = [
        persist.tile([128, NSLOT, TQ], BF16, tag=f"qt{i}", name=f"qt{i}")
        for i in range(2)
    ]
    ots = [
        persist.tile([128, NSLOT, TQ], BF16, tag=f"ot{i}", name=f"ot{i}")
        for i in range(2)
    ]
    # den staging rows (h1=0, h0=32, h2=64) for one batched reciprocal chain
    # per chunk; filler rows hold 1.0 so the chain stays finite on them.
    dent = persist.tile([65, TQ], F32, tag="dent", name="dent")
    lrall = persist.tile([65, TQ], F16, tag="lrall", name="lrall")

    # (KT[0]/KT[1] pad halves need no zeroing: the row-tiled score matmuls
    # contract only each head's 64 data rows)
    nc.gpsimd.memset(Vp01[:, :, :, 0:64], 0.0)
    nc.gpsimd.memset(Vp01[:, :, 0, 32:33], 1.0)
    nc.gpsimd.memset(Vp01[:, :, 1, 0:1], 1.0)
    nc.gpsimd.memset(Vp2[:, :, 64:65], 1.0)
    nc.gpsimd.memset(dent[:], 1.0)
    for i in range(2):
        nc.gpsimd.memset(qts[i][64:128, 1, :], 0.0)
        nc.gpsimd.memset(ots[i][0:64, 1, :], 0.0)



    for j in range(NJ):
        jsl = slice(j * TQ, (j + 1) * TQ)

        # ---- Q/K/V projection for this tq chunk ----
        if j == 0:
            xt = xt0
        else:
            xt = xpool.tile([128, KC, TQ], BF16, tag="xt")
            for kc in range(KC):
                nc.sync.dma_start(xt[:, kc, :], xT_r[:, kc, jsl])
        qt = qts[j % 2]

        # W-chunk order: [Q2|K2] first (K2's shift DMA latency), then
        # [Q0|Q1] (qt slot0 gates h0/h1 score starts), V key-blocks, then
        # [K0|K1] last (K blocks gate only each head's diagonal pairs).
        for m in (2, 0):
            ps = ps_misc.tile([128, TQ], F32, tag="misc", name="psqk")
            for kc in range(KC):
                nc.tensor.matmul(
                    ps[:],
                    lhsT=wqk_sb[:, kc, m * 128:(m + 1) * 128],
                    rhs=xt[:, kc, :],
                    start=(kc == 0),
                    stop=(kc == KC - 1),
                )
            with hp(HP_QKV):
                if m == 0:  # [Q0|Q1] -> qt slot0, one full-height biased copy
                    nc.scalar.activation(
                        out=qt[:, 0, :], in_=ps[:], func=IDENT,
                        bias=bias_sb[:, 0:1],
                    )
                else:  # [Q2|K2]
                    nc.vector.tensor_scalar(
                        out=qt[0:64, 1, :], in0=ps[0:64, :],
                        scalar1=bias_sb[0:64, 2:3], scalar2=None, op0=ADD,
                    )
                    # K2: stage into KT[2] rows 64:128 (qt slot1's zero upper
                    # half makes them don't-care for scores), then partition-
                    # shift down with an SBUF->SBUF DMA.
                    nc.scalar.activation(
                        out=KT[2][64:128, jsl], in_=ps[64:128, :], func=IDENT,
                        bias=bias_sb[64:128, 2:3],
                    )
                    nc.gpsimd.dma_start(KT[2][0:64, jsl], KT[2][64:128, jsl])

        for s in range(4):  # V projection, natural [key, d] layout
            psv = ps_misc.tile([128, TQ], F32, tag="misc", name="psv")
            for kc in range(KC):
                nc.tensor.matmul(
                    psv[:, 0:NV],
                    lhsT=xt[:, kc, s * 128:(s + 1) * 128],
                    rhs=wv_sb[:, kc, :],
                    start=(kc == 0),
                    stop=(kc == KC - 1),
                )
            with hp(HP_QKV):
                nc.vector.tensor_copy(
                    out=Vp01[:, 4 * j + s, :, 64:128], in_=psv[:, 0:128]
                )
                nc.vector.tensor_copy(
                    out=Vp2[:, 4 * j + s, 0:64], in_=psv[:, 128:192]
                )

        ps = ps_misc.tile([128, TQ], F32, tag="misc", name="psk")
        for kc in range(KC):
            nc.tensor.matmul(
                ps[:],
                lhsT=wqk_sb[:, kc, 128:256],
                rhs=xt[:, kc, :],
                start=(kc == 0),
                stop=(kc == KC - 1),
            )
        with hp(HP_QKV):
            nc.vector.tensor_scalar(
                out=KT[0][0:64, jsl], in0=ps[0:64, :],
                scalar1=bias_sb[0:64, 1:2], scalar2=None, op0=ADD,
            )
            nc.scalar.activation(
                out=KT[1][64:128, jsl], in_=ps[64:128, :], func=IDENT,
                bias=bias_sb[64:128, 1:2],
            )

        # ---- streaming attention for this tq chunk ----
        ot = ots[j % 2]
        otmp0 = otmpp.tile([128, TQ], F32, tag="otmp", name="otmp0")
        otmp1 = otmpp.tile([128, TQ], F32, tag="otmp", name="otmp1")
        nblk = 4 * j + 4
        npair = nblk // 2
        def pair_geom(ip):
            i0, i1 = 2 * ip, 2 * ip + 1
            # column trim offsets: block i only contributes to tq columns
            # >= 128*(i-4j) within this chunk
            offs = [max(0, 128 * (i - 4 * j)) for i in (i0, i1)]
            ns = [TQ - o for o in offs]
            return (i0, i1), offs, ns, [0, ns[0]], ns[0] + ns[1]

        def exp_act(pss, w):
            pt = ptp.tile([128, 2 * TQ], BF16, tag="pt")
            nc.scalar.activation(
                out=pt[:, :w], in_=pss[:, :w], func=EXP, scale=SCALE
            )
            return pt

        def exp_act_f(pss, w):
            pt = exp_act(pss, w)
            return lambda s, n: pt[:, s:s + n]

        def exp_sch(pss, w):
            pt16 = ptip.tile([128, 2 * TQ], I16, tag="pt16")
            nc.vector.tensor_scalar(
                out=pt16[:, :w], in0=pss[:, :w],
                scalar1=SCH_A * SCALE / 65536.0, scalar2=SCH_B / 65536.0,
                op0=MULT, op1=ADD,
            )
            ptb = pt16.bitcast(BF16)
            return lambda s, n: ptb[:, s:s + n]

        def exp_diag(pss, w):
            pt = exp_act(pss, w)
            moff = 0 if w == 896 else 896
            nc.vector.tensor_mul(
                pt[:, :w], pt[:, :w], mask_sb[:, moff:moff + w]
            )
            return lambda s, n: pt[:, s:s + n]

        # ---- head 2 alone (qt slot1; upper contraction half is zero) ----
        # software pipeline by one pair: emit scores(ip) one pair ahead of
        # attnV(ip) so the PE always has exp-independent work queued.
        pso2 = ps_o.tile([128, TQ], F32, tag="pso", name="pso2")
        pending = None  # (rhs, offs, ns, starts, (i0, i1))
        for ip in range(npair + 1):
            if ip < npair:
                blks, offs, ns, starts, w = pair_geom(ip)
                pss = ps_pair.tile([128, 2 * TQ], F32, tag="pss")
                for n, i in enumerate(blks):
                    nc.tensor.matmul(
                        pss[:, starts[n]:starts[n] + ns[n]],
                        lhsT=KT[2][:, i * 128:(i + 1) * 128],
                        rhs=qt[:, 1, offs[n]:TQ],
                        start=True,
                        stop=True,
                    )
                if ip - 2 * j >= 0:
                    rhs_of = exp_diag(pss, w)
                elif (ip * SCH_8THS) % 8 < SCH_8THS:
                    rhs_of = exp_sch(pss, w)
                else:
                    rhs_of = exp_act_f(pss, w)
                cur = (rhs_of, offs, ns, starts, blks)
            else:
                cur = None
            if not PIPE_ENABLE:
                pending, cur = cur, None
            if pending is not None:
                (p_rhs, p_offs, p_ns, p_starts, p_blks) = pending
                for n, i in enumerate(p_blks):
                    nc.tensor.matmul(
                        pso2[0:65, p_offs[n]:TQ],
                        lhsT=Vp2[:, i, 0:65],
                        rhs=p_rhs(p_starts[n], p_ns[n]),
                        start=(i == 0),
                        stop=(i == nblk - 1),
                    )
            pending = cur

        if norm_cb is not None:
            # previous chunk's deferred normalize: flushed after h2's pair
            # emission (which covers the chain latency) and before h2's
            # dent write (the chain reads all dent rows).
            norm_cb()
            norm_cb = None
        nc.vector.tensor_copy(otmp0[0:64, :], pso2[0:64, :])
        nc.scalar.copy(dent[64:65, :], pso2[64:65, :])

        # ---- heads 0 and 1 jointly: their K/Q data live in complementary
        # 64-partition halves, so each score pair runs as two concurrent
        # 64-deep row-tiled matmuls (tile rows 0:64 for h0, 64:128 for h1).
        pso0 = ps_o.tile([128, TQ], F32, tag="pso", name="pso0")
        pso1 = ps_o.tile([128, TQ], F32, tag="pso", name="pso1")
        pending = None
        for ip in range(npair + 1):
            if ip < npair:
                blks, offs, ns, starts, w = pair_geom(ip)
                pss0 = ps_pair.tile([128, 2 * TQ], F32, tag="pss", name="pss0")
                pss1 = ps_pair.tile([128, 2 * TQ], F32, tag="pss", name="pss1")
                for n, i in enumerate(blks):
                    nc.tensor.matmul(
                        pss0[:, starts[n]:starts[n] + ns[n]],
                        lhsT=KT[0][0:64, i * 128:(i + 1) * 128],
                        rhs=qt[0:64, 0, offs[n]:TQ],
                        start=True,
                        stop=True,
                    )
                    nc.tensor.matmul(
                        pss1[:, starts[n]:starts[n] + ns[n]],
                        lhsT=KT[1][64:128, i * 128:(i + 1) * 128],
                        rhs=qt[64:128, 0, offs[n]:TQ],
                        start=True,
                        stop=True,
                    )
                if ip - 2 * j >= 0:  # diagonal: exact exp + masks, both heads
                    r0f = exp_diag(pss0, w)
                    r1f = exp_diag(pss1, w)
                elif ip % 2 == 0:  # alternate the approx/exact split per pair
                    r0f, r1f = exp_sch(pss0, w), exp_act_f(pss1, w)
                else:
                    r0f, r1f = exp_act_f(pss0, w), exp_sch(pss1, w)
                cur = ((r0f, r1f), offs, ns, starts, blks)
            else:
                cur = None
            if not PIPE_ENABLE:
                pending, cur = cur, None
            if pending is not None:
                ((p0, p1), p_offs, p_ns, p_starts, p_blks) = pending
                for n, i in enumerate(p_blks):
                    nc.tensor.matmul(
                        pso0[:, p_offs[n]:TQ],
                        lhsT=Vp01[:, i, 0, :],
                        rhs=p0(p_starts[n], p_ns[n]),
                        start=(i == 0),
                        stop=(i == nblk - 1),
                    )
                    nc.tensor.matmul(
                        pso1[:, p_offs[n]:TQ],
                        lhsT=Vp01[:, i, 1, :],
                        rhs=p1(p_starts[n], p_ns[n]),
                        start=(i == 0),
                        stop=(i == nblk - 1),
                    )
            pending = cur

        # copy O rows to SBUF (releases the PSUM banks) and stage den rows
        # into the shared dent tile (h1->0, h0->32, h2->64); the reciprocal
        # chain runs once per chunk, batched.
        nc.vector.tensor_copy(otmp0[64:128, :], pso0[64:128, :])
        nc.scalar.copy(dent[32:33, :], pso0[32:33, :])
        nc.vector.tensor_copy(otmp1[64:128, :], pso1[64:128, :])
        nc.vector.tensor_copy(dent[0:1, :], pso1[0:1, :])

        # deferred normalize for this chunk: one batched reciprocal chain
        # -1/den over dent rows {0,32,64} (filler rows hold 1.0), fp16
        # ones-weighted broadcast matmuls per ot slot, then the final muls.
        def norm_chunk(ot=ot, otmp0=otmp0, otmp1=otmp1):
            with hp(HP_NORM):
                r0i = lrp.tile([65, TQ], I32, tag="r0i")
                nc.vector.tensor_scalar(
                    out=r0i[:], in0=dent[:].bitcast(I32),
                    scalar1=-1, scalar2=RECIP_MAGIC, op0=MULT, op1=ADD,
                )
                r0 = r0i.bitcast(F32)
                td = lrp.tile([65, TQ], F32, tag="td")
                nc.vector.tensor_tensor(
                    out=td[:], in0=dent[:], in1=r0[:], op=MULT,
                )
                nc.vector.scalar_tensor_tensor(
                    out=lrall[:], in0=td[:], scalar=-2.0,
                    in1=r0[:], op0=ADD, op1=MULT,
                )
            psb0 = ps_misc.tile([128, TQ], F32, tag="misc", name="psb0")
            nc.tensor.matmul(
                psb0[:], lhsT=ones_sb[0:65, 0:128], rhs=lrall[:],
                start=True, stop=True,
            )
            with hp(HP_NORM):
                nc.vector.tensor_mul(ot[:, 0, :], otmp0[:], psb0[:])
            psb1 = ps_misc.tile([128, TQ], F32, tag="misc", name="psb1")
            nc.tensor.matmul(
                psb1[64:128, :], lhsT=ones_sb[0:1, 192:256], rhs=lrall[0:1, :],
                start=True, stop=True,
            )
            with hp(HP_NORM):
                nc.vector.tensor_mul(
                    ot[64:128, 1, :], otmp1[64:128, :], psb1[64:128, :]
                )
        norm_cb = norm_chunk

        # ---- output projection, software-pipelined by one chunk ----
        # proj(j-1) is emitted here so the PE has attn(j) work to cover the
        # normalization latency of chunk j-1.
        if j > 0:
            _proj(nc, ps_misc, stp, wp_sb, outT_r, prev_ot, j - 1, nc.gpsimd)
        prev_ot = ot
    norm_cb()
    _proj(nc, ps_misc, stp, wp_sb, outT_r, prev_ot, NJ - 1, nc.sync)


def build_nc():
    nc = bacc.Bacc("TRN2", target_bir_lowering=False, debug=False)
    t = {}
    t["xT"] = nc.dram_tensor("xT", [C, T], BF16, kind="ExternalInput").ap()
    t["wqk"] = nc.dram_tensor("wqk", [C, NQK], BF16, kind="ExternalInput").ap()
    t["wv"] = nc.dram_tensor("wv", [C, NV], BF16, kind="ExternalInput").ap()
    t["bqk"] = nc.dram_tensor("bqk", [3, 128], F32, kind="ExternalInput").ap()
    t["wproj"] = nc.dram_tensor("wproj", [NSLOT, 128, C], BF16, kind="ExternalInput").ap()
    t["masks"] = nc.dram_tensor("masks", [128, 1280], BF16, kind="ExternalInput").ap()
    t["outT"] = nc.dram_tensor("outT", [C, T], BF16, kind="ExternalOutput").ap()
    with tile.TileContext(nc) as tc:
        _mhsa_body(tc, t)
    nc.compile()
    return nc


def make_in_maps(x, W_qkv, b_qkv, W_proj):
    """Shard the full inputs into one input map per core."""
    x = np.asarray(x, dtype=np.float32)
    W_qkv = np.asarray(W_qkv, dtype=np.float32)
    b_qkv = np.asarray(b_qkv, dtype=np.float32)
    W_proj = np.asarray(W_proj, dtype=np.float32)

    q_idx = np.arange(TQ)
    p_idx = np.arange(128)
    m4 = np.zeros((4, 128, TQ), dtype=np.float32)
    for r in range(4):
        m4[r] = (p_idx[:, None] <= (q_idx[None, :] - 128 * r)).astype(np.float32)
    import ml_dtypes
    masks = np.concatenate(
        [m4[0], m4[1][:, 128:], m4[2][:, 256:], m4[3][:, 384:]], axis=1
    ).astype(ml_dtypes.bfloat16)  # [128, 512+384+256+128 = 1280]

    in_maps = []
    for c in range(NCORES):
        b = c // GPB
        g = c % GPB
        heads = [HPC * g + h for h in range(HPC)]

        def wslice(qty, h):
            off = {"Q": 0, "K": C, "V": 2 * C}[qty] + heads[h] * D
            return W_qkv[:, off:off + D]

        def bslice(qty, h):
            off = {"Q": 0, "K": C, "V": 2 * C}[qty] + heads[h] * D
            return b_qkv[off:off + D]

        # chunks: [Q0|Q1], [K0|K1], [Q2|K2]
        wqk = np.concatenate(
            [wslice("Q", 0), wslice("Q", 1),
             wslice("K", 0), wslice("K", 1),
             wslice("Q", 2), wslice("K", 2)], axis=1
        )
        bqk = np.zeros((3, 128), dtype=np.float32)
        bqk[0, 0:64] = bslice("Q", 0)
        bqk[0, 64:128] = bslice("Q", 1)
        bqk[1, 0:64] = bslice("K", 0)
        bqk[1, 64:128] = bslice("K", 1)
        bqk[2, 0:64] = bslice("Q", 2)
        bqk[2, 64:128] = bslice("K", 2)

        wv = np.concatenate([wslice("V", h) for h in range(HPC)], axis=1)

        # negated: the on-core normalize computes -O/den (see kernel comment)
        # slot0 = head2 rows 0:64 + head0 rows 64:128; slot1 = head1 rows 64:128
        wp = np.zeros((NSLOT, 128, C), dtype=np.float32)
        wp[0, 0:64] = -W_proj[heads[2] * D:(heads[2] + 1) * D, :]
        wp[0, 64:128] = -W_proj[heads[0] * D:(heads[0] + 1) * D, :]
        wp[1, 64:128] = -W_proj[heads[1] * D:(heads[1] + 1) * D, :]

        in_maps.append({
            "xT": np.ascontiguousarray(x[b].T).astype(ml_dtypes.bfloat16),
            "wqk": wqk.astype(ml_dtypes.bfloat16),
            "wv": wv.astype(ml_dtypes.bfloat16),
            "bqk": bqk,
            "wproj": wp.astype(ml_dtypes.bfloat16),
            "masks": masks,
        })
    return in_maps


def run_cores(inputs, trace=False, **kw):
    nc = build_nc()
    in_maps = make_in_maps(
        inputs["x"], inputs["W_qkv"], inputs["b_qkv"], inputs["W_proj"]
    )
    res = run_bass_kernel_spmd(nc, in_maps, list(range(NCORES)), trace=trace, **kw)
    return res


def gather(results, b_proj, b_qkv, W_proj):
    b_proj = np.asarray(b_proj, dtype=np.float32)
    b_qkv = np.asarray(b_qkv, dtype=np.float32)
    W_proj = np.asarray(W_proj, dtype=np.float32)
    # V bias commutes through softmax: out += b_v @ W_proj (host-side)
    b_eff = b_proj + b_qkv[2 * C:3 * C] @ W_proj
    out = np.zeros((B, T, C), dtype=np.float32)
    for c in range(NCORES):
        out[c // GPB] += results[c]["outT"].astype(np.float32).T
    out += b_eff
    return out


def kernel(x, W_qkv, b_qkv, W_proj, b_proj):
    res = run_cores(
        {"x": x, "W_qkv": W_qkv, "b_qkv": b_qkv, "W_proj": W_proj}
    )
    return gather(res.results, b_proj, b_qkv, W_proj)
